# revision 30
# baseline (speedup 1.0000x reference)
"""Trainium2 Bass kernel for fused attention (QKV proj + RoPE + SDPA + o_proj).

Sharding: Megatron-style tensor parallel over heads (4 heads/core x 8 cores)
for QKV+SDPA, then per-(batch, query-half) AllToAll rounds switch to token
parallelism for o_proj, so each core emits a disjoint slice of the output.

Design (iterated against neuron-profile traces; ~764us -> ~580-600us):
  - all matmuls bf16 (psum accumulates f32); w_o resident in SBUF (64KB/p)
  - ap=512 moving rows; RoPE batched per [128,512] psum tile with the
    rotate-half partition swap done by 4 small SBUF-SBUF DMAs
  - the tensor queue is kept dense: o_proj of round b-1 AND the entire
    QKV projection of batch b+1 are generators fed into batch b's SDPA
    kt-loop (emitted between the exps and the PV matmuls, so interleaved
    matmuls fill the exp-wait window and keep the PE p-state hot)
  - per-batch AllToAll rounds overlap the next batch's compute; batch 3
    is split into two query-half rounds and its o_proj halves overlap
    the final exchange; asl tiles are DMA'd on the gpsimd queue right
    after each collective fires; a warmup collective absorbs cc setup
  - softmax: ones-column in V gives denominators from the PV matmul;
    reciprocal_approx_fast + DRAM-bounce broadcast; exp outputs bf16
  - engine balance: Act only exps; Vector rope math + psum drains +
    normalize; Sync all data DMA issue; GpSimd collectives + asl loads
  - PSUM: 2 banks proj / 2 o_proj / 2 scores / 2 attn-out = 8
"""
import sys

import numpy as np

try:
    import concourse.bass as bass
except ImportError:  # fresh grading env: make the toolchain importable
    for p in (
        "/root/.axon_site",
        "/root/.axon_site/_ro/trn_rl_repo",
        "/root/.axon_site/_ro/pypackages",
        "/opt/trn_rl_repo",
        "/opt/pypackages",
    ):
        if p not in sys.path:
            sys.path.append(p)
    import concourse.bass as bass

import concourse.bacc as bacc
import concourse.mybir as mybir
import concourse.tile as tile
from concourse.bass_utils import run_bass_kernel_spmd

import ml_dtypes

F32 = mybir.dt.float32
F32R = mybir.dt.float32r
BF16 = mybir.dt.bfloat16
MULT = mybir.AluOpType.mult
ADD = mybir.AluOpType.add
EXP = mybir.ActivationFunctionType.Exp
IDENT = mybir.ActivationFunctionType.Identity

# problem dims (hardcoded for nn_Attention_42846593744909)
B, S, D = 4, 1024, 2048
H, HD = 32, 64
N_CORES = 8
H_LOC = H // N_CORES  # heads per core


def build_attention(b=B, s=S, d=D, h_loc=H_LOC, hd=HD, n_cores=N_CORES):
    """Build the per-core SPMD Bass program. Returns finalized nc."""
    P = 128
    T = b * s                  # total tokens
    DCH = d // P               # contraction chunks for D (16)
    QBLK = h_loc * hd          # 256
    NQK = 2 * QBLK // P        # q+k e-chunks (4)
    EVA = h_loc * (hd + 1)     # v + ones columns (260)
    TH = 512                   # proj token half-batch
    NTH = s // TH              # 2
    QT = 512                   # query tile in SDPA
    NQT = s // QT              # 2
    KTC = s // P               # key chunks of 128 (8)
    ECH = n_cores * QBLK // P  # o_proj contraction chunks (16)
    RT = s // n_cores          # tokens per core per batch round (128)
    ODC = 512                  # o_proj dout chunk (psum bank)
    NDC = d // ODC             # 4
    TS = b * RT                # output tokens per core (512)

    nc = bacc.Bacc()
    hidden_t = nc.dram_tensor("hidden_t", [d, T], BF16, kind="ExternalInput")
    w_qk_t = nc.dram_tensor("w_qk_t", [d, 2 * QBLK], BF16, kind="ExternalInput")
    w_v_t = nc.dram_tensor("w_v_t", [d, QBLK], BF16, kind="ExternalInput")
    w_o_t = nc.dram_tensor("w_o_t", [n_cores * QBLK, d], BF16, kind="ExternalInput")
    cos2 = nc.dram_tensor("cos2", [P, s], F32, kind="ExternalInput")
    sinrot2 = nc.dram_tensor("sinrot2", [P, s], F32, kind="ExternalInput")
    out_sl = nc.dram_tensor("out_sl", [TS, d], F32, kind="ExternalOutput")

    hid_v = hidden_t[:].rearrange("(c p) t -> p c t", p=P)
    wqk_v = w_qk_t[:].rearrange("(c p) e -> p c e", p=P)
    wv_v = w_v_t[:].rearrange("(c p) e -> p c e", p=P)
    wo_v = w_o_t[:].rearrange("(c p) e -> p c e", p=P)

    with tile.TileContext(nc) as tc:
        with tc.tile_pool(name="dramp", bufs=1, space="DRAM") as dramp:
            ccw_in = dramp.tile([n_cores, 1, 64], BF16, name="ccw_in")
            ccw_out = dramp.tile([n_cores, 1, 64], BF16, name="ccw_out")
            cc_in = [dramp.tile([n_cores, QBLK, RT], BF16, name=f"cc_in_{r}")
                     for r in range(b - 1)]
            cc_out = [dramp.tile([n_cores, QBLK, RT], BF16, name=f"cc_out_{r}")
                      for r in range(b - 1)]
            # last batch split into qt halves for a shorter tail
            cc3_in = [dramp.tile([n_cores, QBLK, RT // 2], BF16, name=f"cc3_in_{q}")
                      for q in range(NQT)]
            cc3_out = [dramp.tile([n_cores, QBLK, RT // 2], BF16, name=f"cc3_out_{q}")
                       for q in range(NQT)]
            # scatter view: [pp, h, v, dst core j, t]
            ccin_v = [t_[:].rearrange("j (pp h v) t -> pp h v j t", pp=2, h=2, v=hd)
                      for t_ in cc_in]
            ccout_v = [t_[:].rearrange("j (ci p) t -> p (j ci) t", p=P)
                       for t_ in cc_out]
            cc3in_v = [t_[:].rearrange("j (pp h v) t -> pp h v j t", pp=2, h=2, v=hd)
                       for t_ in cc3_in]
            cc3out_v = [t_[:].rearrange("j (ci p) t -> p (j ci) t", p=P)
                        for t_ in cc3_out]

            with (
                tc.tile_pool(name="tabs", bufs=1) as tabs,
                tc.tile_pool(name="hidp", bufs=2) as hidp,
                tc.tile_pool(name="qkp", bufs=2) as qkp,
                tc.tile_pool(name="vp", bufs=2) as vp,
                tc.tile_pool(name="ropep", bufs=2) as ropep,
                tc.tile_pool(name="expp", bufs=2) as expp,
                tc.tile_pool(name="aop", bufs=2) as aop,
                tc.tile_pool(name="dgp", bufs=1) as dgp,
                tc.tile_pool(name="repp", bufs=1) as repp,
                tc.tile_pool(name="aonp", bufs=1) as aonp,
                tc.tile_pool(name="aslp", bufs=2) as aslp,
                tc.tile_pool(name="vfp", bufs=2) as vfp,
                tc.tile_pool(name="obp", bufs=1) as obp,
                tc.tile_pool(name="drowp", bufs=4, space="DRAM") as drowp,
                tc.tile_pool(name="psP", bufs=2, space="PSUM") as psP,
                tc.tile_pool(name="psJ", bufs=2, space="PSUM") as psJ,
                tc.tile_pool(name="psS", bufs=2, space="PSUM") as psS,
                tc.tile_pool(name="psO", bufs=2, space="PSUM") as psO,
            ):
                # ---- static tables / weights; issue order = priority.
                # Big loads spread across sync/scalar/gpsimd direct-DMA
                # queues so transfers run in parallel.
                wqk_sb = tabs.tile([P, DCH, 2 * QBLK], BF16)
                nc.sync.dma_start(wqk_sb[:, :, 0:P], wqk_v[:, :, 0:P])

                def load_hid_half(bi, th, eng=None):
                    t0 = bi * s + th * TH
                    hid_sb = hidp.tile([P, DCH, TH], BF16, tag="hid", name="hid")
                    (eng or nc.sync).dma_start(hid_sb[:], hid_v[:, :, t0:t0 + TH])
                    return hid_sb

                cos_sb = tabs.tile([P, s], F32)
                sin_sb = tabs.tile([P, s], F32)
                wv_sb = tabs.tile([P, DCH, QBLK], BF16)
                hid0 = hidp.tile([P, DCH, TH], BF16, tag="hid", name="hid")
                nc.scalar.dma_start(wv_sb[:], wv_v[:])
                for g in range(4):  # chunked: first MMs start after chunk 0
                    nc.sync.dma_start(hid0[:, 4 * g:4 * g + 4, :],
                                      hid_v[:, 4 * g:4 * g + 4, 0:TH])
                nc.sync.dma_start(cos_sb[:], cos2[:])
                nc.sync.dma_start(sin_sb[:], sinrot2[:])
                # warmup collective: absorb cc stream setup off critical path
                nc.gpsimd.collective_compute(
                    "AllToAll", mybir.AluOpType.bypass,
                    replica_groups=[list(range(n_cores))],
                    ins=[ccw_in.opt()], outs=[ccw_out.opt()])
                for ec in range(1, NQK):
                    nc.sync.dma_start(wqk_sb[:, :, ec * P:(ec + 1) * P],
                                      wqk_v[:, :, ec * P:(ec + 1) * P])
                hid1 = load_hid_half(0, 1)
                hid_next = [hid0, hid1]
                wo_sb = tabs.tile([P, ECH, d], BF16)
                for dc in range(8):
                    nc.scalar.dma_start(wo_sb[:, :, dc * 256:(dc + 1) * 256],
                                        wo_v[:, :, dc * 256:(dc + 1) * 256])

                def rope(ps, soff, qk_t, ec):
                    """RoPE a [128, TH] psum tile into qk_t[:, ec, soff:soff+TH]."""
                    raw = ropep.tile([P, TH], F32, tag="raw", name="raw")
                    nc.vector.tensor_copy(raw[:], ps[:])
                    cp = ropep.tile([P, TH], F32, tag="cp", name="cp", bufs=1)
                    nc.vector.tensor_tensor(cp[:], raw[:], cos_sb[:, soff:soff + TH], MULT)
                    sw = ropep.tile([P, TH], F32, tag="sw", name="sw", bufs=1)
                    # rotate_half: swap 32-partition blocks within each head
                    nc.sync.dma_start(sw[0:32, :], raw[32:64, :])
                    nc.sync.dma_start(sw[32:64, :], raw[0:32, :])
                    nc.sync.dma_start(sw[64:96, :], raw[96:128, :])
                    nc.sync.dma_start(sw[96:128, :], raw[64:96, :])
                    nc.vector.tensor_tensor(sw[:], sw[:], sin_sb[:, soff:soff + TH], MULT)
                    nc.vector.tensor_tensor(qk_t[:, ec, soff:soff + TH], cp[:], sw[:], ADD)

                hid_store = {}

                def proj_gen(bi, qk_t, v_t):
                    """QKV projection + RoPE for batch bi, in 17 steps."""
                    hid_tiles = hid_store.pop(bi)
                    for h in range(h_loc):
                        nc.scalar.activation(
                            v_t[:, :, h * (hd + 1) + hd:h * (hd + 1) + hd + 1],
                            wv_sb[:, 0:KTC, 0:1], IDENT, bias=1.0, scale=0.0)
                    yield
                    for th in range(NTH):
                        s0 = th * TH
                        hid_sb = hid_tiles[th]
                        for ec in range(NQK):
                            ps = psP.tile([P, ODC], F32, tag="psP", name="psqk")
                            for dd in range(DCH):
                                nc.tensor.matmul(
                                    ps[:], lhsT=wqk_sb[:, dd, ec * P:(ec + 1) * P],
                                    rhs=hid_sb[:, dd, :],
                                    start=(dd == 0), stop=(dd == DCH - 1))
                            rope(ps, s0, qk_t, ec)
                            yield
                        for tsub in range(TH // P):
                            kc = th * (TH // P) + tsub
                            psv = psP.tile([P, ODC], F32, tag="psP", name="psv")
                            for dd in range(DCH):
                                nc.tensor.matmul(
                                    psv[:, 0:QBLK],
                                    lhsT=hid_sb[:, dd, tsub * P:(tsub + 1) * P],
                                    rhs=wv_sb[:, dd, :],
                                    start=(dd == 0), stop=(dd == DCH - 1))
                            for h in range(h_loc):
                                nc.vector.tensor_copy(
                                    v_t[:, kc, h * (hd + 1):h * (hd + 1) + hd],
                                    psv[:, h * hd:(h + 1) * hd])
                            yield
                        # prefetch next batch's same-half activations
                        if bi + 1 < b:
                            hid_store.setdefault(bi + 1, []).append(
                                load_hid_half(bi + 1, th))

                def oproj_gen(r, pace=5):
                    """Generator emitting o_proj for round r."""
                    asl = aslp.tile([P, ECH, RT], BF16, tag="asl", name=f"asl{r}")
                    nc.gpsimd.dma_start(asl[:], ccout_v[r])
                    for _ in range(pace):
                        yield  # pacing: let round r's A2A land first
                    for dcg in range(NDC // 2):
                        pj0 = psJ.tile([P, ODC], F32, tag="psJ", name="pj0")
                        pj1 = psJ.tile([P, ODC], F32, tag="psJ", name="pj1")
                        d0 = (2 * dcg) * ODC
                        d1 = (2 * dcg + 1) * ODC
                        for e2 in range(ECH // 2):
                            for e in (2 * e2, 2 * e2 + 1):
                                nc.tensor.matmul(pj0[:], lhsT=asl[:, e, :],
                                                 rhs=wo_sb[:, e, d0:d0 + ODC],
                                                 start=(e == 0), stop=(e == ECH - 1))
                                nc.tensor.matmul(pj1[:], lhsT=asl[:, e, :],
                                                 rhs=wo_sb[:, e, d1:d1 + ODC],
                                                 start=(e == 0), stop=(e == ECH - 1))
                            yield
                        ob = obp.tile([P, 2 * ODC], F32, tag="ob", name="ob")
                        nc.vector.tensor_copy(ob[:, 0:ODC], pj0[:])
                        nc.vector.tensor_copy(ob[:, ODC:2 * ODC], pj1[:])
                        nc.sync.dma_start(
                            out_sl[r * RT:(r + 1) * RT, d0:d0 + 2 * ODC], ob[:])
                        yield

                def a2a(r):
                    nc.gpsimd.collective_compute(
                        "AllToAll", mybir.AluOpType.bypass,
                        replica_groups=[list(range(n_cores))],
                        ins=[cc_in[r].opt()], outs=[cc_out[r].opt()])

                def sdpa(bi, qk_t, v_t, feeder, pgen):
                    """SDPA for batch bi; o_proj of round bi-1 and the
                    projection of batch bi+1 interleave into the slots."""
                    slot = 0
                    for qt in range(NQT):
                        q0 = qt * QT
                        for pp in range(h_loc // 2):
                            ps_o0 = psO.tile([P, QT], F32, tag="psO", name="pso0")
                            ps_o1 = psO.tile([P, QT], F32, tag="psO", name="pso1")
                            for kt in range(KTC):
                                ps_s0 = psS.tile([P, QT], F32, tag="psS", name="pss0")
                                ps_s1 = psS.tile([P, QT], F32, tag="psS", name="pss1")
                                nc.tensor.matmul(
                                    ps_s0[:],
                                    lhsT=qk_t[0:64, 2 + pp, kt * P:(kt + 1) * P],
                                    rhs=qk_t[0:64, pp, q0:q0 + QT],
                                    start=True, stop=True)
                                nc.tensor.matmul(
                                    ps_s1[:],
                                    lhsT=qk_t[64:128, 2 + pp, kt * P:(kt + 1) * P],
                                    rhs=qk_t[64:128, pp, q0:q0 + QT],
                                    start=True, stop=True, tile_position=(64, 0))
                                e0 = expp.tile([P, QT], BF16, tag="exp", name="e0")
                                e1 = expp.tile([P, QT], BF16, tag="exp", name="e1")
                                nc.scalar.activation(e0[:], ps_s0[:], EXP)
                                nc.scalar.activation(e1[:], ps_s1[:], EXP)
                                # interleaved work fills the exp-wait window
                                slot += 1
                                if slot > 8:  # let round bi-1's A2A land first
                                    next(feeder, None)
                                if slot % 2 == 0:
                                    next(pgen, None)
                                h0 = 2 * pp
                                h1 = 2 * pp + 1
                                nc.tensor.matmul(
                                    ps_o0[0:hd + 1, :],
                                    lhsT=v_t[:, kt, h0 * (hd + 1):(h0 + 1) * (hd + 1)],
                                    rhs=e0[:],
                                    start=(kt == 0), stop=(kt == KTC - 1))
                                nc.tensor.matmul(
                                    ps_o1[0:hd + 1, :],
                                    lhsT=v_t[:, kt, h1 * (hd + 1):(h1 + 1) * (hd + 1)],
                                    rhs=e1[:],
                                    start=(kt == 0), stop=(kt == KTC - 1))
                            # stash to SBUF, free psum
                            ao = aop.tile([hd + 1, 2, QT], F32, tag="ao", name="ao")
                            nc.vector.tensor_copy(ao[:, 0, :], ps_o0[0:hd + 1, :])
                            nc.vector.tensor_copy(ao[:, 1, :], ps_o1[0:hd + 1, :])
                            # softmax denominators -> 1/den, broadcast via DRAM
                            dg = dgp.tile([2, QT], F32, tag="dg", name="dg")
                            nc.sync.dma_start(dg[:], ao[hd:hd + 1, :, :])
                            rcp = dgp.tile([2, QT], F32, tag="rcp", name="rcp")
                            nc.vector.reciprocal_approx_fast(rcp[:], dg[:])
                            rd = drowp.tile([2, QT], F32, tag="drow", name="rd")
                            nc.sync.dma_start(rd[:], rcp[:])
                            rep = repp.tile([hd, 2, QT], F32, tag="rep", name="rep")
                            nc.sync.dma_start(rep[:, 0, :],
                                              rd[0:1, :].to_broadcast((hd, QT)))
                            nc.sync.dma_start(rep[:, 1, :],
                                              rd[1:2, :].to_broadcast((hd, QT)))
                            aon = aonp.tile([hd, 2, QT], BF16, tag="aon", name="aon")
                            nc.vector.tensor_tensor(aon[:], ao[0:hd, :, :], rep[:], MULT)
                            # scatter (one DMA per head)
                            if bi < b - 1:
                                j0 = qt * (QT // RT)
                                for h in range(2):
                                    nc.sync.dma_start(
                                        ccin_v[bi][pp, h][:, j0:j0 + QT // RT, :],
                                        aon[:, h, :].rearrange(
                                            "v (j t) -> v j t", t=RT))
                            else:
                                for h in range(2):
                                    nc.sync.dma_start(
                                        cc3in_v[qt][pp, h],
                                        aon[:, h, :].rearrange(
                                            "v (j t) -> v j t", t=RT // 2))
                        if bi == b - 1:  # fire this qt-half's exchange now
                            nc.gpsimd.collective_compute(
                                "AllToAll", mybir.AluOpType.bypass,
                                replica_groups=[list(range(n_cores))],
                                ins=[cc3_in[qt].opt()], outs=[cc3_out[qt].opt()])
                            a3 = aslp.tile([P, ECH, RT // 2], BF16, tag="asl3",
                                           name=f"asl3_{qt}")
                            nc.gpsimd.dma_start(a3[:], cc3out_v[qt])
                            asl3_tiles[qt] = a3
                    if bi < b - 1:
                        a2a(bi)

                def empty_gen():
                    return iter(())

                asl3_tiles = {}

                feeder = empty_gen()
                hid_store[0] = hid_next
                qk_t = qkp.tile([P, NQK, s], BF16, tag="qk", name="qk")
                v_t = vp.tile([P, KTC, EVA], BF16, tag="v", name="v")
                for _ in proj_gen(0, qk_t, v_t):  # batch 0: run fully
                    pass
                for bi in range(b):
                    if bi + 1 < b:
                        qk_n = qkp.tile([P, NQK, s], BF16, tag="qk", name="qk")
                        v_n = vp.tile([P, KTC, EVA], BF16, tag="v", name="v")
                        pgen = proj_gen(bi + 1, qk_n, v_n)
                    else:
                        pgen = empty_gen()
                    sdpa(bi, qk_t, v_t, feeder, pgen)
                    for _ in feeder:  # drain leftover o_proj steps
                        pass
                    for _ in pgen:  # drain leftover projection steps
                        pass
                    if bi < b - 1:
                        feeder = oproj_gen(bi, pace=8 if bi == 0 else 5)
                        next(feeder, None)  # emit asl DMA (gpsimd) eagerly
                        qk_t, v_t = qk_n, v_n
                    else:
                        feeder = empty_gen()
                # last batch o_proj: per qt-half, first overlaps second's A2A
                def oproj3(q):
                    asl3 = asl3_tiles[q]
                    for dcg in range(NDC // 2):
                        pj0 = psJ.tile([P, ODC], F32, tag="psJ", name="pj0")
                        pj1 = psJ.tile([P, ODC], F32, tag="psJ", name="pj1")
                        d0 = (2 * dcg) * ODC
                        d1 = (2 * dcg + 1) * ODC
                        for e in range(ECH):
                            nc.tensor.matmul(pj0[0:RT // 2, :], lhsT=asl3[:, e, :],
                                             rhs=wo_sb[:, e, d0:d0 + ODC],
                                             start=(e == 0), stop=(e == ECH - 1))
                            nc.tensor.matmul(pj1[0:RT // 2, :], lhsT=asl3[:, e, :],
                                             rhs=wo_sb[:, e, d1:d1 + ODC],
                                             start=(e == 0), stop=(e == ECH - 1))
                        ob = obp.tile([P, 2 * ODC], F32, tag="ob", name="ob")
                        nc.vector.tensor_copy(ob[0:RT // 2, 0:ODC], pj0[0:RT // 2, :])
                        nc.vector.tensor_copy(ob[0:RT // 2, ODC:2 * ODC],
                                              pj1[0:RT // 2, :])
                        r0 = (b - 1) * RT + q * (RT // 2)
                        nc.sync.dma_start(
                            out_sl[r0:r0 + RT // 2, d0:d0 + 2 * ODC],
                            ob[0:RT // 2, :])

                for q in range(NQT):
                    oproj3(q)
    nc.finalize()
    return nc


def prep_inputs(cos, sin, hidden_states, w_qkv, w_o,
                b=B, s=S, d=D, h_loc=H_LOC, hd=HD, n_cores=N_CORES):
    """Host-side sharding/layout: returns per-core input maps."""
    BF = ml_dtypes.bfloat16
    cos = np.asarray(cos, dtype=np.float32)
    sin = np.asarray(sin, dtype=np.float32)
    hidden_states = np.asarray(hidden_states, dtype=np.float32)
    w_qkv = np.asarray(w_qkv, dtype=np.float32)
    w_o = np.asarray(w_o, dtype=np.float32)

    T = b * s
    QBLK = h_loc * hd
    HHD = n_cores * QBLK  # total H*HD

    hidden_t = np.ascontiguousarray(hidden_states.reshape(T, d).T).astype(BF)
    w_o_t = np.ascontiguousarray(w_o.T).astype(BF)

    ident2_np = np.ascontiguousarray(
        np.vstack([np.eye(hd), np.eye(hd)]).astype(np.float32))
    cos_t = cos.T  # [hd, s]
    sin_t = sin.T
    cos2 = np.ascontiguousarray(np.tile(cos_t, (128 // hd, 1)))
    srt = sin_t.copy()
    srt[0:hd // 2] = -sin_t[0:hd // 2]
    sinrot2 = np.ascontiguousarray(np.tile(srt, (128 // hd, 1)))

    maps = []
    for c in range(n_cores):
        wq = w_qkv[c * QBLK:(c + 1) * QBLK] * 0.125
        wk = w_qkv[HHD + c * QBLK:HHD + (c + 1) * QBLK]
        wv = w_qkv[2 * HHD + c * QBLK:2 * HHD + (c + 1) * QBLK]
        w_qk_t = np.ascontiguousarray(np.concatenate([wq, wk], axis=0).T).astype(BF)
        w_v_t = np.ascontiguousarray(wv.T).astype(BF)
        maps.append({
            "hidden_t": hidden_t,
            "w_qk_t": w_qk_t,
            "w_v_t": w_v_t,
            "w_o_t": w_o_t,
            "cos2": cos2,
            "sinrot2": sinrot2,
        })
    return maps


_NC_CACHE = {}


def run(inputs, trace=False, dims=None):
    """Run the distributed kernel. Returns (full_output, BassKernelResults)."""
    dims = dims or dict(b=B, s=S, d=D, h_loc=H_LOC, hd=HD, n_cores=N_CORES)
    key = tuple(sorted(dims.items()))
    if key not in _NC_CACHE:
        _NC_CACHE[key] = build_attention(**dims)
    nc = _NC_CACHE[key]
    maps = prep_inputs(inputs["cos"], inputs["sin"], inputs["hidden_states"],
                       inputs["w_qkv"], inputs["w_o"], **dims)
    res = run_bass_kernel_spmd(nc, maps, list(range(dims["n_cores"])), trace=trace)
    n_cores = dims["n_cores"]
    s = dims["s"]
    RT = s // n_cores  # 128
    T = dims["b"] * s
    out = np.empty((T, dims["d"]), dtype=np.float32)
    for c in range(n_cores):
        sl = res.results[c]["out_sl"]
        for r in range(dims["b"] - 1):
            out[r * s + c * RT: r * s + (c + 1) * RT] = sl[r * RT:(r + 1) * RT]
    # last batch was exchanged in qt halves of 64 tokens per core
    r = dims["b"] - 1
    for c in range(n_cores):
        sl = res.results[c]["out_sl"]
        for q in range(2):
            g0 = r * s + q * 512 + c * (RT // 2)
            o0 = r * RT + q * (RT // 2)
            out[g0:g0 + RT // 2] = sl[o0:o0 + RT // 2]
    out = out.reshape(dims["b"], s, dims["d"])
    return out, res


def kernel(**inputs) -> np.ndarray:
    out, _ = run(inputs)
    return out


# revision 31
# speedup vs baseline: 1.0315x; 1.0315x over previous
"""Trainium2 Bass kernel for fused attention (QKV proj + RoPE + SDPA + o_proj).

Sharding: Megatron-style tensor parallel over heads (4 heads/core x 8 cores)
for QKV+SDPA, then per-(batch, query-half) AllToAll rounds switch to token
parallelism for o_proj, so each core emits a disjoint slice of the output.

v3 design:
  - all matmuls bf16 (psum accumulate f32); w_o resident in SBUF
  - ap=512 moving rows everywhere; RoPE batched per [128,512] tile
  - 8 small AllToAll rounds (one per batch x query-half), fired as soon as
    their tokens are normalized; warmup collective absorbs stream setup
  - o_proj matmuls interleaved into the SDPA loop (SDPA is Act-limited)
  - engine balance: Act does only exp; Vector does rope math + psum
    drains; Sync does all DMA issue; GpSimd only triggers collectives
"""
import sys

import numpy as np

try:
    import concourse.bass as bass
except ImportError:  # fresh grading env: make the toolchain importable
    for p in (
        "/root/.axon_site",
        "/root/.axon_site/_ro/trn_rl_repo",
        "/root/.axon_site/_ro/pypackages",
        "/opt/trn_rl_repo",
        "/opt/pypackages",
    ):
        if p not in sys.path:
            sys.path.append(p)
    import concourse.bass as bass

import concourse.bacc as bacc
import concourse.mybir as mybir
import concourse.tile as tile
from concourse.bass_utils import run_bass_kernel_spmd

import ml_dtypes

F32 = mybir.dt.float32
F32R = mybir.dt.float32r
BF16 = mybir.dt.bfloat16
MULT = mybir.AluOpType.mult
ADD = mybir.AluOpType.add
EXP = mybir.ActivationFunctionType.Exp
IDENT = mybir.ActivationFunctionType.Identity

# problem dims (hardcoded for nn_Attention_42846593744909)
B, S, D = 4, 1024, 2048
H, HD = 32, 64
N_CORES = 8
H_LOC = H // N_CORES  # heads per core


def build_attention(b=B, s=S, d=D, h_loc=H_LOC, hd=HD, n_cores=N_CORES):
    """Build the per-core SPMD Bass program. Returns finalized nc."""
    P = 128
    T = b * s                  # total tokens
    DCH = d // P               # contraction chunks for D (16)
    QBLK = h_loc * hd          # 256
    NQK = 2 * QBLK // P        # q+k e-chunks (4)
    EVA = h_loc * (hd + 1)     # v + ones columns (260)
    TH = 512                   # proj token half-batch
    NTH = s // TH              # 2
    QT = 512                   # query tile in SDPA
    NQT = s // QT              # 2
    KTC = s // P               # key chunks of 128 (8)
    ECH = n_cores * QBLK // P  # o_proj contraction chunks (16)
    RT = s // n_cores          # tokens per core per batch round (128)
    ODC = 512                  # o_proj dout chunk (psum bank)
    NDC = d // ODC             # 4
    TS = b * RT                # output tokens per core (512)

    nc = bacc.Bacc()
    hidden_t = nc.dram_tensor("hidden_t", [d, T], BF16, kind="ExternalInput")
    w_qk_t = nc.dram_tensor("w_qk_t", [d, 2 * QBLK], BF16, kind="ExternalInput")
    w_v_t = nc.dram_tensor("w_v_t", [d, QBLK], BF16, kind="ExternalInput")
    w_o_t = nc.dram_tensor("w_o_t", [n_cores * QBLK, d], BF16, kind="ExternalInput")
    cos2 = nc.dram_tensor("cos2", [P, s], F32, kind="ExternalInput")
    sinrot2 = nc.dram_tensor("sinrot2", [P, s], F32, kind="ExternalInput")
    out_sl = nc.dram_tensor("out_sl", [TS, d], F32, kind="ExternalOutput")

    hid_v = hidden_t[:].rearrange("(c p) t -> p c t", p=P)
    wqk_v = w_qk_t[:].rearrange("(c p) e -> p c e", p=P)
    wv_v = w_v_t[:].rearrange("(c p) e -> p c e", p=P)
    wo_v = w_o_t[:].rearrange("(c p) e -> p c e", p=P)

    with tile.TileContext(nc) as tc:
        with tc.tile_pool(name="dramp", bufs=1, space="DRAM") as dramp:
            ccw_in = dramp.tile([n_cores, 1, 64], BF16, name="ccw_in")
            ccw_out = dramp.tile([n_cores, 1, 64], BF16, name="ccw_out")
            cc_in = [dramp.tile([n_cores, QBLK, RT], BF16, name=f"cc_in_{r}")
                     for r in range(b - 1)]
            cc_out = [dramp.tile([n_cores, QBLK, RT], BF16, name=f"cc_out_{r}")
                      for r in range(b - 1)]
            # last batch split into qt halves for a shorter tail
            cc3_in = [dramp.tile([n_cores, QBLK, RT // 2], BF16, name=f"cc3_in_{q}")
                      for q in range(NQT)]
            cc3_out = [dramp.tile([n_cores, QBLK, RT // 2], BF16, name=f"cc3_out_{q}")
                       for q in range(NQT)]
            # scatter view: [pp, h, v, dst core j, t]
            ccin_v = [t_[:].rearrange("j (pp h v) t -> pp h v j t", pp=2, h=2, v=hd)
                      for t_ in cc_in]
            ccout_v = [t_[:].rearrange("j (ci p) t -> p (j ci) t", p=P)
                       for t_ in cc_out]
            cc3in_v = [t_[:].rearrange("j (pp h v) t -> pp h v j t", pp=2, h=2, v=hd)
                       for t_ in cc3_in]
            cc3out_v = [t_[:].rearrange("j (ci p) t -> p (j ci) t", p=P)
                        for t_ in cc3_out]

            with (
                tc.tile_pool(name="tabs", bufs=1) as tabs,
                tc.tile_pool(name="hidp", bufs=2) as hidp,
                tc.tile_pool(name="qkp", bufs=2) as qkp,
                tc.tile_pool(name="vp", bufs=2) as vp,
                tc.tile_pool(name="ropep", bufs=2) as ropep,
                tc.tile_pool(name="expp", bufs=2) as expp,
                tc.tile_pool(name="aop", bufs=2) as aop,
                tc.tile_pool(name="dgp", bufs=1) as dgp,
                tc.tile_pool(name="repp", bufs=1) as repp,
                tc.tile_pool(name="aonp", bufs=1) as aonp,
                tc.tile_pool(name="aslp", bufs=2) as aslp,
                tc.tile_pool(name="vfp", bufs=2) as vfp,
                tc.tile_pool(name="obp", bufs=1) as obp,
                tc.tile_pool(name="drowp", bufs=4, space="DRAM") as drowp,
                tc.tile_pool(name="psP", bufs=2, space="PSUM") as psP,
                tc.tile_pool(name="psJ", bufs=2, space="PSUM") as psJ,
                tc.tile_pool(name="psS", bufs=2, space="PSUM") as psS,
                tc.tile_pool(name="psO", bufs=2, space="PSUM") as psO,
            ):
                # ---- static tables / weights; issue order = priority.
                # Big loads spread across sync/scalar/gpsimd direct-DMA
                # queues so transfers run in parallel.
                wqk_sb = tabs.tile([P, DCH, 2 * QBLK], BF16)
                nc.sync.dma_start(wqk_sb[:, :, 0:P], wqk_v[:, :, 0:P])

                def load_hid_half(bi, th, eng=None):
                    t0 = bi * s + th * TH
                    hid_sb = hidp.tile([P, DCH, TH], BF16, tag="hid", name="hid")
                    (eng or nc.sync).dma_start(hid_sb[:], hid_v[:, :, t0:t0 + TH])
                    return hid_sb

                cos_sb = tabs.tile([P, s], F32)
                sin_sb = tabs.tile([P, s], F32)
                wv_sb = tabs.tile([P, DCH, QBLK], BF16)
                hid0 = hidp.tile([P, DCH, TH], BF16, tag="hid", name="hid")
                for g in range(4):  # chunked: first MMs start after chunk 0
                    nc.sync.dma_start(hid0[:, 4 * g:4 * g + 4, :],
                                      hid_v[:, 4 * g:4 * g + 4, 0:TH])
                nc.scalar.dma_start(cos_sb[:], cos2[:])
                nc.scalar.dma_start(sin_sb[:], sinrot2[:])
                nc.scalar.dma_start(wv_sb[:], wv_v[:])
                # warmup collective: absorb cc stream setup off critical path
                nc.gpsimd.collective_compute(
                    "AllToAll", mybir.AluOpType.bypass,
                    replica_groups=[list(range(n_cores))],
                    ins=[ccw_in.opt()], outs=[ccw_out.opt()])
                for ec in range(1, NQK):
                    nc.sync.dma_start(wqk_sb[:, :, ec * P:(ec + 1) * P],
                                      wqk_v[:, :, ec * P:(ec + 1) * P])
                hid1 = load_hid_half(0, 1)
                hid_next = [hid0, hid1]
                wo_sb = tabs.tile([P, ECH, d], BF16)
                for dc in range(8):
                    nc.scalar.dma_start(wo_sb[:, :, dc * 256:(dc + 1) * 256],
                                        wo_v[:, :, dc * 256:(dc + 1) * 256])

                def rope(ps, soff, qk_t, ec):
                    """RoPE a [128, TH] psum tile into qk_t[:, ec, soff:soff+TH]."""
                    raw = ropep.tile([P, TH], F32, tag="raw", name="raw")
                    nc.vector.tensor_copy(raw[:], ps[:])
                    cp = ropep.tile([P, TH], F32, tag="cp", name="cp", bufs=1)
                    nc.vector.tensor_tensor(cp[:], raw[:], cos_sb[:, soff:soff + TH], MULT)
                    sw = ropep.tile([P, TH], F32, tag="sw", name="sw", bufs=1)
                    # rotate_half: swap 32-partition blocks within each head
                    nc.sync.dma_start(sw[0:32, :], raw[32:64, :])
                    nc.sync.dma_start(sw[32:64, :], raw[0:32, :])
                    nc.sync.dma_start(sw[64:96, :], raw[96:128, :])
                    nc.sync.dma_start(sw[96:128, :], raw[64:96, :])
                    nc.vector.tensor_tensor(sw[:], sw[:], sin_sb[:, soff:soff + TH], MULT)
                    nc.vector.tensor_tensor(qk_t[:, ec, soff:soff + TH], cp[:], sw[:], ADD)

                hid_store = {}

                def proj_gen(bi, qk_t, v_t):
                    """QKV projection + RoPE for batch bi, in 17 steps."""
                    hid_tiles = hid_store.pop(bi)
                    for h in range(h_loc):
                        nc.scalar.activation(
                            v_t[:, :, h * (hd + 1) + hd:h * (hd + 1) + hd + 1],
                            wv_sb[:, 0:KTC, 0:1], IDENT, bias=1.0, scale=0.0)
                    yield
                    for th in range(NTH):
                        s0 = th * TH
                        hid_sb = hid_tiles[th]
                        for ec in range(NQK):
                            ps = psP.tile([P, ODC], F32, tag="psP", name="psqk")
                            for dd in range(DCH):
                                nc.tensor.matmul(
                                    ps[:], lhsT=wqk_sb[:, dd, ec * P:(ec + 1) * P],
                                    rhs=hid_sb[:, dd, :],
                                    start=(dd == 0), stop=(dd == DCH - 1))
                            rope(ps, s0, qk_t, ec)
                            yield
                        for tsub in range(TH // P):
                            kc = th * (TH // P) + tsub
                            psv = psP.tile([P, ODC], F32, tag="psP", name="psv")
                            for dd in range(DCH):
                                nc.tensor.matmul(
                                    psv[:, 0:QBLK],
                                    lhsT=hid_sb[:, dd, tsub * P:(tsub + 1) * P],
                                    rhs=wv_sb[:, dd, :],
                                    start=(dd == 0), stop=(dd == DCH - 1))
                            for h in range(h_loc):
                                nc.vector.tensor_copy(
                                    v_t[:, kc, h * (hd + 1):h * (hd + 1) + hd],
                                    psv[:, h * hd:(h + 1) * hd])
                            yield
                        # prefetch next batch's same-half activations
                        if bi + 1 < b:
                            hid_store.setdefault(bi + 1, []).append(
                                load_hid_half(bi + 1, th))

                def oproj_gen(r, pace=5):
                    """Generator emitting o_proj for round r."""
                    asl = aslp.tile([P, ECH, RT], BF16, tag="asl", name=f"asl{r}")
                    nc.gpsimd.dma_start(asl[:], ccout_v[r])
                    for _ in range(pace):
                        yield  # pacing: let round r's A2A land first
                    for dcg in range(NDC // 2):
                        pj0 = psJ.tile([P, ODC], F32, tag="psJ", name="pj0")
                        pj1 = psJ.tile([P, ODC], F32, tag="psJ", name="pj1")
                        d0 = (2 * dcg) * ODC
                        d1 = (2 * dcg + 1) * ODC
                        for e2 in range(ECH // 2):
                            for e in (2 * e2, 2 * e2 + 1):
                                nc.tensor.matmul(pj0[:], lhsT=asl[:, e, :],
                                                 rhs=wo_sb[:, e, d0:d0 + ODC],
                                                 start=(e == 0), stop=(e == ECH - 1))
                                nc.tensor.matmul(pj1[:], lhsT=asl[:, e, :],
                                                 rhs=wo_sb[:, e, d1:d1 + ODC],
                                                 start=(e == 0), stop=(e == ECH - 1))
                            yield
                        ob = obp.tile([P, 2 * ODC], F32, tag="ob", name="ob")
                        nc.vector.tensor_copy(ob[:, 0:ODC], pj0[:])
                        nc.vector.tensor_copy(ob[:, ODC:2 * ODC], pj1[:])
                        nc.sync.dma_start(
                            out_sl[r * RT:(r + 1) * RT, d0:d0 + 2 * ODC], ob[:])
                        yield

                def a2a(r):
                    nc.gpsimd.collective_compute(
                        "AllToAll", mybir.AluOpType.bypass,
                        replica_groups=[list(range(n_cores))],
                        ins=[cc_in[r].opt()], outs=[cc_out[r].opt()])

                def sdpa(bi, qk_t, v_t, feeder, pgen):
                    """SDPA for batch bi; o_proj of round bi-1 and the
                    projection of batch bi+1 interleave into the slots."""
                    slot = 0
                    for qt in range(NQT):
                        q0 = qt * QT
                        for pp in range(h_loc // 2):
                            ps_o0 = psO.tile([P, QT], F32, tag="psO", name="pso0")
                            ps_o1 = psO.tile([P, QT], F32, tag="psO", name="pso1")
                            for kt in range(KTC):
                                ps_s0 = psS.tile([P, QT], F32, tag="psS", name="pss0")
                                ps_s1 = psS.tile([P, QT], F32, tag="psS", name="pss1")
                                nc.tensor.matmul(
                                    ps_s0[:],
                                    lhsT=qk_t[0:64, 2 + pp, kt * P:(kt + 1) * P],
                                    rhs=qk_t[0:64, pp, q0:q0 + QT],
                                    start=True, stop=True)
                                nc.tensor.matmul(
                                    ps_s1[:],
                                    lhsT=qk_t[64:128, 2 + pp, kt * P:(kt + 1) * P],
                                    rhs=qk_t[64:128, pp, q0:q0 + QT],
                                    start=True, stop=True, tile_position=(64, 0))
                                e0 = expp.tile([P, QT], BF16, tag="exp", name="e0")
                                e1 = expp.tile([P, QT], BF16, tag="exp", name="e1")
                                nc.scalar.activation(e0[:], ps_s0[:], EXP)
                                nc.scalar.activation(e1[:], ps_s1[:], EXP)
                                # interleaved work fills the exp-wait window
                                slot += 1
                                if slot > 8:  # let round bi-1's A2A land first
                                    next(feeder, None)
                                if slot % 2 == 0:
                                    next(pgen, None)
                                h0 = 2 * pp
                                h1 = 2 * pp + 1
                                nc.tensor.matmul(
                                    ps_o0[0:hd + 1, :],
                                    lhsT=v_t[:, kt, h0 * (hd + 1):(h0 + 1) * (hd + 1)],
                                    rhs=e0[:],
                                    start=(kt == 0), stop=(kt == KTC - 1))
                                nc.tensor.matmul(
                                    ps_o1[0:hd + 1, :],
                                    lhsT=v_t[:, kt, h1 * (hd + 1):(h1 + 1) * (hd + 1)],
                                    rhs=e1[:],
                                    start=(kt == 0), stop=(kt == KTC - 1))
                            # stash to SBUF, free psum
                            ao = aop.tile([hd + 1, 2, QT], F32, tag="ao", name="ao")
                            nc.vector.tensor_copy(ao[:, 0, :], ps_o0[0:hd + 1, :])
                            nc.vector.tensor_copy(ao[:, 1, :], ps_o1[0:hd + 1, :])
                            # softmax denominators -> 1/den, broadcast via DRAM
                            dg = dgp.tile([2, QT], F32, tag="dg", name="dg")
                            nc.sync.dma_start(dg[:], ao[hd:hd + 1, :, :])
                            rcp = dgp.tile([2, QT], F32, tag="rcp", name="rcp")
                            nc.vector.reciprocal_approx_fast(rcp[:], dg[:])
                            rd = drowp.tile([2, QT], F32, tag="drow", name="rd")
                            nc.sync.dma_start(rd[:], rcp[:])
                            rep = repp.tile([hd, 2, QT], F32, tag="rep", name="rep")
                            nc.sync.dma_start(rep[:, 0, :],
                                              rd[0:1, :].to_broadcast((hd, QT)))
                            nc.sync.dma_start(rep[:, 1, :],
                                              rd[1:2, :].to_broadcast((hd, QT)))
                            aon = aonp.tile([hd, 2, QT], BF16, tag="aon", name="aon")
                            nc.vector.tensor_tensor(aon[:], ao[0:hd, :, :], rep[:], MULT)
                            # scatter (one DMA per head)
                            if bi < b - 1:
                                j0 = qt * (QT // RT)
                                for h in range(2):
                                    nc.sync.dma_start(
                                        ccin_v[bi][pp, h][:, j0:j0 + QT // RT, :],
                                        aon[:, h, :].rearrange(
                                            "v (j t) -> v j t", t=RT))
                            else:
                                for h in range(2):
                                    nc.sync.dma_start(
                                        cc3in_v[qt][pp, h],
                                        aon[:, h, :].rearrange(
                                            "v (j t) -> v j t", t=RT // 2))
                        if bi == b - 1:  # fire this qt-half's exchange now
                            nc.gpsimd.collective_compute(
                                "AllToAll", mybir.AluOpType.bypass,
                                replica_groups=[list(range(n_cores))],
                                ins=[cc3_in[qt].opt()], outs=[cc3_out[qt].opt()])
                    if bi < b - 1:
                        a2a(bi)

                def empty_gen():
                    return iter(())

                feeder = empty_gen()
                hid_store[0] = hid_next
                qk_t = qkp.tile([P, NQK, s], BF16, tag="qk", name="qk")
                v_t = vp.tile([P, KTC, EVA], BF16, tag="v", name="v")
                for _ in proj_gen(0, qk_t, v_t):  # batch 0: run fully
                    pass
                for bi in range(b):
                    if bi + 1 < b:
                        qk_n = qkp.tile([P, NQK, s], BF16, tag="qk", name="qk")
                        v_n = vp.tile([P, KTC, EVA], BF16, tag="v", name="v")
                        pgen = proj_gen(bi + 1, qk_n, v_n)
                    else:
                        pgen = empty_gen()
                    sdpa(bi, qk_t, v_t, feeder, pgen)
                    for _ in feeder:  # drain leftover o_proj steps
                        pass
                    for _ in pgen:  # drain leftover projection steps
                        pass
                    if bi < b - 1:
                        feeder = oproj_gen(bi, pace=8 if bi == 0 else 5)
                        next(feeder, None)  # emit asl DMA (gpsimd) eagerly
                        qk_t, v_t = qk_n, v_n
                    else:
                        feeder = empty_gen()
                # last batch o_proj: per qt-half, first overlaps second's A2A
                def oproj3(q):
                    asl3 = aslp.tile([P, ECH, RT // 2], BF16, tag="asl3",
                                     name=f"asl3_{q}")
                    nc.gpsimd.dma_start(asl3[:], cc3out_v[q])
                    for dcg in range(NDC // 2):
                        pj0 = psJ.tile([P, ODC], F32, tag="psJ", name="pj0")
                        pj1 = psJ.tile([P, ODC], F32, tag="psJ", name="pj1")
                        d0 = (2 * dcg) * ODC
                        d1 = (2 * dcg + 1) * ODC
                        for e in range(ECH):
                            nc.tensor.matmul(pj0[0:RT // 2, :], lhsT=asl3[:, e, :],
                                             rhs=wo_sb[:, e, d0:d0 + ODC],
                                             start=(e == 0), stop=(e == ECH - 1))
                            nc.tensor.matmul(pj1[0:RT // 2, :], lhsT=asl3[:, e, :],
                                             rhs=wo_sb[:, e, d1:d1 + ODC],
                                             start=(e == 0), stop=(e == ECH - 1))
                        ob = obp.tile([P, 2 * ODC], F32, tag="ob", name="ob")
                        nc.vector.tensor_copy(ob[0:RT // 2, 0:ODC], pj0[0:RT // 2, :])
                        nc.vector.tensor_copy(ob[0:RT // 2, ODC:2 * ODC],
                                              pj1[0:RT // 2, :])
                        r0 = (b - 1) * RT + q * (RT // 2)
                        nc.sync.dma_start(
                            out_sl[r0:r0 + RT // 2, d0:d0 + 2 * ODC],
                            ob[0:RT // 2, :])

                for q in range(NQT):
                    oproj3(q)
    nc.finalize()
    return nc


def prep_inputs(cos, sin, hidden_states, w_qkv, w_o,
                b=B, s=S, d=D, h_loc=H_LOC, hd=HD, n_cores=N_CORES):
    """Host-side sharding/layout: returns per-core input maps."""
    BF = ml_dtypes.bfloat16
    cos = np.asarray(cos, dtype=np.float32)
    sin = np.asarray(sin, dtype=np.float32)
    hidden_states = np.asarray(hidden_states, dtype=np.float32)
    w_qkv = np.asarray(w_qkv, dtype=np.float32)
    w_o = np.asarray(w_o, dtype=np.float32)

    T = b * s
    QBLK = h_loc * hd
    HHD = n_cores * QBLK  # total H*HD

    hidden_t = np.ascontiguousarray(hidden_states.reshape(T, d).T).astype(BF)
    w_o_t = np.ascontiguousarray(w_o.T).astype(BF)

    ident2_np = np.ascontiguousarray(
        np.vstack([np.eye(hd), np.eye(hd)]).astype(np.float32))
    cos_t = cos.T  # [hd, s]
    sin_t = sin.T
    cos2 = np.ascontiguousarray(np.tile(cos_t, (128 // hd, 1)))
    srt = sin_t.copy()
    srt[0:hd // 2] = -sin_t[0:hd // 2]
    sinrot2 = np.ascontiguousarray(np.tile(srt, (128 // hd, 1)))

    maps = []
    for c in range(n_cores):
        wq = w_qkv[c * QBLK:(c + 1) * QBLK] * 0.125
        wk = w_qkv[HHD + c * QBLK:HHD + (c + 1) * QBLK]
        wv = w_qkv[2 * HHD + c * QBLK:2 * HHD + (c + 1) * QBLK]
        w_qk_t = np.ascontiguousarray(np.concatenate([wq, wk], axis=0).T).astype(BF)
        w_v_t = np.ascontiguousarray(wv.T).astype(BF)
        maps.append({
            "hidden_t": hidden_t,
            "w_qk_t": w_qk_t,
            "w_v_t": w_v_t,
            "w_o_t": w_o_t,
            "cos2": cos2,
            "sinrot2": sinrot2,
        })
    return maps


_NC_CACHE = {}


def run(inputs, trace=False, dims=None):
    """Run the distributed kernel. Returns (full_output, BassKernelResults)."""
    dims = dims or dict(b=B, s=S, d=D, h_loc=H_LOC, hd=HD, n_cores=N_CORES)
    key = tuple(sorted(dims.items()))
    if key not in _NC_CACHE:
        _NC_CACHE[key] = build_attention(**dims)
    nc = _NC_CACHE[key]
    maps = prep_inputs(inputs["cos"], inputs["sin"], inputs["hidden_states"],
                       inputs["w_qkv"], inputs["w_o"], **dims)
    res = run_bass_kernel_spmd(nc, maps, list(range(dims["n_cores"])), trace=trace)
    n_cores = dims["n_cores"]
    s = dims["s"]
    RT = s // n_cores  # 128
    T = dims["b"] * s
    out = np.empty((T, dims["d"]), dtype=np.float32)
    for c in range(n_cores):
        sl = res.results[c]["out_sl"]
        for r in range(dims["b"] - 1):
            out[r * s + c * RT: r * s + (c + 1) * RT] = sl[r * RT:(r + 1) * RT]
    # last batch was exchanged in qt halves of 64 tokens per core
    r = dims["b"] - 1
    for c in range(n_cores):
        sl = res.results[c]["out_sl"]
        for q in range(2):
            g0 = r * s + q * 512 + c * (RT // 2)
            o0 = r * RT + q * (RT // 2)
            out[g0:g0 + RT // 2] = sl[o0:o0 + RT // 2]
    out = out.reshape(dims["b"], s, dims["d"])
    return out, res


def kernel(**inputs) -> np.ndarray:
    out, _ = run(inputs)
    return out
